# revision 28
# baseline (speedup 1.0000x reference)
"""GalerkinBlock forward on 8 trn2 NeuronCores (Bass/Tile).

Sharding: data-parallel over tokens. Core c handles batch b=c//2, sequence
half c%2 (4096 tokens each). The linear-attention context k^T v is a sum
over tokens, so each core accumulates a partial [H,D,D] context and
all-reduces it with its pair (replica groups [[0,1],[2,3],[4,5],[6,7]]).

Per-core layout strategy (P = 128 partitions):
  - activations stream token-major ("T": [128 tokens, channels]) for all
    layernorms (bn_stats over the free axis) and channel-major
    ("C": [128 channels, tokens]) for matmul contractions; T->C conversion
    uses PE transpose-mode matmuls into PSUM + one ACT copy per sub-block.
  - All matmuls run in bf16 with fp32 PSUM accumulation.
  - LN affine folds: norm1 w/b folded into qkv_w/qkv_b, norm2 into mlp_w1,
    attention SCALE folded into proj_w (all host-side, exact).
  - attn+proj fused: x1 = x + sum_h q_h @ Watt_h with
    Watt_h = ctx_h @ (SCALE*proj_w_h) computed once after the all-reduce.
  - Both passes are software-pipelined one block ahead (the DVE/ACT
    layernorm chains for block b+1 are emitted before block b's heavy
    matmuls so the PE never starves).
"""

import os
import sys

import numpy as np

for _p in ("/opt/trn_rl_repo",):
    if os.path.isdir(_p) and _p not in sys.path:
        sys.path.insert(0, _p)

import ml_dtypes
import concourse.bass as bass
import concourse.tile as tile
from concourse import bacc, mybir
from concourse.bass_utils import run_bass_kernel_spmd
from concourse.masks import make_identity

F32 = mybir.dt.float32
BF16 = mybir.dt.bfloat16
AF = mybir.ActivationFunctionType
ALU = mybir.AluOpType

B, N, C = 4, 8192, 512
H, D = 4, 128
HID = 4 * C
EPS = 1e-5
SCALE = D ** -0.5
NCORES = 8
TB = 512          # tokens per block
SUBS = TB // 128  # 128-token sub-blocks per block

# set by the last run when KERNEL_TRACE=1
last_exec_time_ns = None
last_profile = None


def _bcast_ap(src_ap, parts, reps):
    """[K]-shaped DRAM AP -> [parts, reps*K] broadcast AP (partition step 0)."""
    ap = [[0, parts]]
    if reps > 1:
        ap.append([0, reps])
    ap.extend(src_ap.ap)
    return bass.AP(tensor=src_ap.tensor, offset=src_ap.offset, ap=ap)


def build_program(T=N // 2, kv_bias=False, kv_affine=False, proj_bias=False,
                  mlp2_bias=False, q_bias=False):
    assert T % TB == 0
    NB = T // TB
    nc = bacc.Bacc("TRN2", target_bir_lowering=False, debug=False,
                   num_devices=NCORES)

    xs = nc.dram_tensor("xs", [T, C], F32, kind="ExternalInput")
    wqkv = nc.dram_tensor("wqkv", [4, 128, 3 * C], BF16, kind="ExternalInput")
    wproj = nc.dram_tensor("wproj", [H, 128, C], BF16, kind="ExternalInput")
    wm1 = nc.dram_tensor("wm1", [4, 128, HID], BF16, kind="ExternalInput")
    wm2 = nc.dram_tensor("wm2", [16, 128, C], BF16, kind="ExternalInput")
    bq = nc.dram_tensor("bq", [128, H], F32, kind="ExternalInput")
    bm1 = nc.dram_tensor("bm1", [128, 16], F32, kind="ExternalInput")
    bkv = nc.dram_tensor("bkv", [1, 2 * C], BF16, kind="ExternalInput")
    lnkv = nc.dram_tensor("lnkv", [4, 128], F32, kind="ExternalInput")
    bpb2 = nc.dram_tensor("bpb2", [2, C], F32, kind="ExternalInput")
    out = nc.dram_tensor("out", [T, C], F32, kind="ExternalOutput")

    rg = [[0, 1], [2, 3], [4, 5], [6, 7]]

    with tile.TileContext(nc) as tc:
        from contextlib import ExitStack
        with ExitStack() as outer:
            singles = outer.enter_context(tc.tile_pool(name="singles", bufs=1))
            xpool = outer.enter_context(tc.tile_pool(name="xin", bufs=3))
            zpool = outer.enter_context(tc.tile_pool(name="zt", bufs=10))
            stats = outer.enter_context(tc.tile_pool(name="stats", bufs=4))
            trpool = outer.enter_context(
                tc.tile_pool(name="trp", bufs=1, space="PSUM"))
            dpool = outer.enter_context(
                tc.tile_pool(name="dram", bufs=1, space="DRAM"))

            # ---- persistent weights / constants ----
            # small constants first: the identity matrix gates block 0's PE
            # transposes, so it must not queue behind megabytes of weights
            eps_t = singles.tile([128, 1], F32)
            nc.vector.memset(eps_t[:], EPS)
            ident = singles.tile([128, 128], BF16)
            make_identity(nc, ident[:])
            bq_sb = singles.tile([128, H], F32)
            nc.gpsimd.dma_start(bq_sb[:], bq[:])
            bm1_sb = singles.tile([128, 16], F32)
            nc.gpsimd.dma_start(bm1_sb[:], bm1[:])
            wqkv_sb = singles.tile([128, 4, 3 * C], BF16)
            nc.gpsimd.dma_start(wqkv_sb[:, :, C:3 * C],
                                wqkv[:, :, C:3 * C].rearrange("a p n -> p a n"))
            nc.gpsimd.dma_start(wqkv_sb[:, :, 0:C],
                                wqkv[:, :, 0:C].rearrange("a p n -> p a n"))
            wproj_sb = singles.tile([128, H, C], BF16)
            nc.gpsimd.dma_start(wproj_sb[:], wproj[:].rearrange("a p n -> p a n"))
            wm1_sb = singles.tile([128, 4, HID], BF16)
            nc.gpsimd.dma_start(wm1_sb[:], wm1[:].rearrange("a p n -> p a n"))
            wm2_sb = singles.tile([128, 16, C], BF16)
            nc.gpsimd.dma_start(wm2_sb[:], wm2[:].rearrange("a p n -> p a n"))

            q_all = singles.tile([128, H, T], BF16)

            if kv_bias:
                bkv_sb = singles.tile([1, 2 * C], BF16)
                nc.gpsimd.dma_start(bkv_sb[:], bkv[:])
                ones1 = singles.tile([1, 128], BF16)
                nc.vector.memset(ones1[:], 1.0)
            if kv_affine:
                aff = []
                for r in range(4):
                    t = singles.tile([128, 4, 128], F32, tag=f"aff{r}",
                                     name=f"aff{r}")
                    nc.gpsimd.dma_start(t[:], _bcast_ap(lnkv[r, :], 128, 4))
                    aff.append(t)
                wk4, bk4, wv4, bv4 = aff
            if proj_bias:
                bp_sb = singles.tile([128, C], F32)
                nc.gpsimd.dma_start(bp_sb[:], _bcast_ap(bpb2[0, :], 128, 1))
            if mlp2_bias:
                b2_sb = singles.tile([128, C], F32)
                nc.gpsimd.dma_start(b2_sb[:], _bcast_ap(bpb2[1, :], 128, 1))

            def layernorm_stats(x_ap):
                """x_ap [128, C] f32 -> (mu [128,1], rsig [128,1])"""
                st6 = stats.tile([128, 6], F32, tag="st6", name="st6")
                nc.vector.bn_stats(st6[:], x_ap)
                mv = stats.tile([128, 2], F32, tag="mv", name="mv")
                nc.vector.bn_aggr(mv[:], st6[:])
                sd = stats.tile([128, 1], F32, tag="sd", name="sd")
                nc.scalar.activation(sd[:], mv[:, 1:2], AF.Sqrt, bias=eps_t[:])
                rs = stats.tile([128, 1], F32, tag="rs", name="rs")
                nc.vector.reciprocal(rs[:], sd[:])
                return mv[:, 0:1], rs[:]

            def transpose_sub(zt, zC, sub):
                """PE-transpose [128tok, 512ch] -> zC[:, :, sub, :]"""
                tr = trpool.tile([128, 4, 128], BF16, tag="trp", name="tr")
                for kc in range(4):
                    nc.tensor.transpose(tr[:, kc, :],
                                        zt[:, kc * 128:(kc + 1) * 128],
                                        ident[:])
                nc.scalar.activation(zC[:, :, sub, :], tr[:], AF.Copy)

            # ================= PASS 1: qkv + context =================
            # q matmuls for the last N_DEFER blocks are deferred to after the
            # context all-reduce trigger, so the collective's latency hides
            # under ~N_DEFER*3.5us of pure PE work.
            N_DEFER = NB
            with ExitStack() as p1:
                zcpool = p1.enter_context(
                    tc.tile_pool(name="zc", bufs=N_DEFER + 2))
                zkvpool = p1.enter_context(tc.tile_pool(name="zkv", bufs=4))
                # PSUM pools for the kv phase live in their own stack so the
                # deferred-q phase can reuse their banks
                p1a = ExitStack()
                ctxpool = p1a.enter_context(
                    tc.tile_pool(name="ctxps", bufs=1, space="PSUM"))
                kvps = p1a.enter_context(
                    tc.tile_pool(name="kvps", bufs=3, space="PSUM"))

                ctx_ps = ctxpool.tile([128, H, 128], F32)

                def ln1_block(blk):
                    xt = xpool.tile([128, SUBS, C], F32, tag="xt",
                                    name=f"xt{blk}")
                    nc.sync.dma_start(
                        xt[:],
                        xs[blk * TB:(blk + 1) * TB, :].rearrange(
                            "(s p) c -> p s c", p=128))
                    zts = []
                    for sub in range(SUBS):
                        mu, rs = layernorm_stats(xt[:, sub, :])
                        zt = zpool.tile([128, C], BF16, tag="zt",
                                        name=f"zt{blk}_{sub}")
                        nc.vector.tensor_scalar(
                            zt[:], xt[:, sub, :], mu, rs,
                            op0=ALU.subtract, op1=ALU.mult)
                        zts.append(zt)
                    return zts

                last_ctx_mm = [None]

                def emit_ctx(zk, zv, blk, sub):
                    for h in range(H):
                        first = (blk == 0 and sub == 0 and h == 0)
                        last = (blk == NB - 1 and sub == SUBS - 1
                                and h == H - 1)
                        last_ctx_mm[0] = nc.tensor.matmul(
                            ctx_ps[:, h, :],
                            zv[:, h * 128:(h + 1) * 128],
                            zk[:, h * 128:(h + 1) * 128],
                            start=first, stop=last)

                def emit_q(blk, zC, after=None):
                    for mc in range(H):
                        q_ps = qps.tile([128, TB], F32, tag="qps", name="qps")
                        for kc in range(4):
                            mm = nc.tensor.matmul(
                                q_ps[:],
                                wqkv_sb[:, kc, mc * 128:(mc + 1) * 128],
                                zC[:, kc, :, :], start=(kc == 0),
                                stop=(kc == 3))
                            if after is not None:
                                tile.add_dep_helper(
                                    mm.ins, after.ins, sync=False,
                                    reason="defer q past ctx accumulation")
                        if q_bias:
                            nc.scalar.activation(
                                q_all[:, mc, blk * TB:(blk + 1) * TB],
                                q_ps[:], AF.Identity,
                                bias=bq_sb[:, mc:mc + 1])
                        else:
                            nc.scalar.activation(
                                q_all[:, mc, blk * TB:(blk + 1) * TB],
                                q_ps[:], AF.Copy)

                zts_next = ln1_block(0)
                deferred_q = []
                pending = []
                for blk in range(NB):
                    zts = zts_next
                    if blk + 1 < NB:
                        zts_next = ln1_block(blk + 1)
                    zC = zcpool.tile([128, 4, SUBS, 128], BF16, tag="zC",
                                     name=f"zC{blk}")
                    for sub in range(SUBS):
                        transpose_sub(zts[sub], zC, sub)
                    # q: weight-stationary, C-major out [d, tok]
                    if blk < NB - N_DEFER:
                        emit_q(blk, zC)
                    else:
                        deferred_q.append((blk, zC))
                    # k,v: x-stationary, T-major out [tok, ch]; the ctx
                    # accumulation for sub s is emitted during sub s+2 so the
                    # PE isn't gated on s's LN-stats chain (~6us deep).
                    for sub in range(SUBS):
                        k_ps = kvps.tile([128, C], F32, tag="kps", name="kps")
                        v_ps = kvps.tile([128, C], F32, tag="vps", name="vps")
                        for kc in range(4):
                            zsub = zC[:, kc, sub, :]
                            nc.tensor.matmul(
                                k_ps[:], zsub, wqkv_sb[:, kc, C:2 * C],
                                start=(kc == 0),
                                stop=(kc == 3 and not kv_bias))
                            nc.tensor.matmul(
                                v_ps[:], zsub, wqkv_sb[:, kc, 2 * C:3 * C],
                                start=(kc == 0),
                                stop=(kc == 3 and not kv_bias))
                        if kv_bias:
                            nc.tensor.matmul(k_ps[:], ones1[:],
                                             bkv_sb[:, 0:C],
                                             start=False, stop=True)
                            nc.tensor.matmul(v_ps[:], ones1[:],
                                             bkv_sb[:, C:2 * C],
                                             start=False, stop=True)
                        if len(pending) >= 2:
                            emit_ctx(*pending.pop(0))
                        zk = zkvpool.tile([128, C], BF16, tag="zk", name="zk")
                        zv = zkvpool.tile([128, C], BF16, tag="zv", name="zv")
                        for ps, z in ((k_ps, zk), (v_ps, zv)):
                            mv = stats.tile([128, 4, 2], F32, tag="kvmv",
                                            name="kvmv")
                            for h in range(H):
                                st6 = stats.tile([128, 6], F32, tag="kvst6",
                                                 name="kvst6")
                                nc.vector.bn_stats(
                                    st6[:], ps[:, h * 128:(h + 1) * 128])
                                nc.vector.bn_aggr(mv[:, h, :], st6[:])
                            sd = stats.tile([128, 4], F32, tag="kvsd",
                                            name="kvsd")
                            nc.scalar.activation(sd[:], mv[:, :, 1], AF.Sqrt,
                                                 bias=eps_t[:])
                            rs = stats.tile([128, 4], F32, tag="kvrs",
                                            name="kvrs")
                            nc.vector.reciprocal(rs[:], sd[:])
                            nmrs = stats.tile([128, 4], F32, tag="kvnm",
                                              name="kvnm")
                            nc.vector.tensor_tensor(nmrs[:], mv[:, :, 0],
                                                    rs[:], op=ALU.mult)
                            nc.vector.tensor_scalar_mul(nmrs[:], nmrs[:], -1.0)
                            for h in range(H):
                                nc.scalar.activation(
                                    z[:, h * 128:(h + 1) * 128],
                                    ps[:, h * 128:(h + 1) * 128],
                                    AF.Identity, bias=nmrs[:, h:h + 1],
                                    scale=rs[:, h:h + 1])
                        if kv_affine:
                            for z, w4, b4 in ((zk, wk4, bk4), (zv, wv4, bv4)):
                                zf = z[:].rearrange("p (h d) -> p h d", d=128)
                                nc.vector.tensor_tensor(zf, zf, w4[:],
                                                        op=ALU.mult)
                                nc.vector.tensor_tensor(zf, zf, b4[:],
                                                        op=ALU.add)
                        pending.append((zk, zv, blk, sub))
                while pending:
                    emit_ctx(*pending.pop(0))

                ctx_sb = singles.tile([128, H * 128], BF16)
                nc.vector.tensor_copy(
                    ctx_sb[:], ctx_ps[:].rearrange("p h d -> p (h d)"))

                # launch the all-reduce (bf16: CCE adds in bf16, halves the
                # wire bytes), then run the deferred q matmuls under it
                cc_in = dpool.tile([128, H * 128], BF16)
                cc_out = dpool.tile([128, H * 128], BF16)
                nc.gpsimd.dma_start(cc_in[:], ctx_sb[:])
                nc.gpsimd.collective_compute(
                    "AllReduce", ALU.add, replica_groups=rg,
                    ins=[cc_in[:].opt()], outs=[cc_out[:].opt()])
                p1a.close()
                p1b = ExitStack()
                qps = p1b.enter_context(
                    tc.tile_pool(name="qps", bufs=2, space="PSUM"))
                for blk, zC in deferred_q:
                    emit_q(blk, zC, after=last_ctx_mm[0])
                p1b.close()

            # ================= PASS 2: attn+proj, mlp =================
            with ExitStack() as p2:
                z2cpool = p2.enter_context(tc.tile_pool(name="z2c", bufs=2))
                x1pool = p2.enter_context(tc.tile_pool(name="x1", bufs=9))
                midpool = p2.enter_context(tc.tile_pool(name="mid", bufs=2))
                outpool = p2.enter_context(tc.tile_pool(name="outp", bufs=2))
                xps = p2.enter_context(
                    tc.tile_pool(name="xps", bufs=2, space="PSUM"))
                mps = p2.enter_context(
                    tc.tile_pool(name="mps", bufs=2, space="PSUM"))
                ops = p2.enter_context(
                    tc.tile_pool(name="ops", bufs=2, space="PSUM"))

                # Watt_h = ctx_h @ (SCALE * proj_w_h)  -> [d, C] bf16
                watt_sb = singles.tile([128, H, C], BF16)
                ctxr_bf = singles.tile([128, H * 128], BF16)
                nc.gpsimd.dma_start(ctxr_bf[:], cc_out[:])
                for h in range(H):
                    w_ps = xps.tile([128, C], F32, tag="xps", name="wps")
                    nc.tensor.matmul(w_ps[:],
                                     ctxr_bf[:, h * 128:(h + 1) * 128],
                                     wproj_sb[:, h, :], start=True, stop=True)
                    nc.scalar.activation(watt_sb[:, h, :], w_ps[:], AF.Copy)

                def attn_chain(blk):
                    """attn+proj matmuls, residual, LN2 -> (x1s, z2ts)"""
                    xt = xpool.tile([128, SUBS, C], F32, tag="xt",
                                    name=f"x2t{blk}")
                    nc.sync.dma_start(
                        xt[:],
                        xs[blk * TB:(blk + 1) * TB, :].rearrange(
                            "(s p) c -> p s c", p=128))
                    x1s, z2ts = [], []
                    for sub in range(SUBS):
                        x1_ps = xps.tile([128, C], F32, tag="xps", name="xps")
                        t0 = blk * TB + sub * 128
                        for h in range(H):
                            nc.tensor.matmul(
                                x1_ps[:], q_all[:, h, t0:t0 + 128],
                                watt_sb[:, h, :], start=(h == 0),
                                stop=(h == 3))
                        x1t = x1pool.tile([128, C], F32, tag="x1",
                                          name=f"x1_{blk}_{sub}")
                        nc.vector.tensor_tensor(x1t[:], x1_ps[:],
                                                xt[:, sub, :], op=ALU.add)
                        if proj_bias:
                            nc.vector.tensor_tensor(x1t[:], x1t[:], bp_sb[:],
                                                    op=ALU.add)
                        mu, rs = layernorm_stats(x1t[:])
                        z2t = zpool.tile([128, C], BF16, tag="zt",
                                         name=f"z2t{blk}_{sub}")
                        nc.vector.tensor_scalar(
                            z2t[:], x1t[:], mu, rs,
                            op0=ALU.subtract, op1=ALU.mult)
                        x1s.append(x1t)
                        z2ts.append(z2t)
                    return x1s, z2ts

                def tr2_block(z2ts, blk):
                    z2C = z2cpool.tile([128, 4, SUBS, 128], BF16, tag="z2C",
                                       name=f"z2C{blk}")
                    for sub in range(SUBS):
                        transpose_sub(z2ts[sub], z2C, sub)
                    return z2C

                x1s_cur, z2ts_cur = attn_chain(0)
                z2C_cur = tr2_block(z2ts_cur, 0)
                for blk in range(NB):
                    x1s, z2C = x1s_cur, z2C_cur
                    if blk + 1 < NB:
                        x1s_cur, z2ts_cur = attn_chain(blk + 1)
                    mid = midpool.tile([128, 16, TB], BF16, tag="mid",
                                       name=f"mid{blk}")
                    for mc in range(16):
                        m_ps = mps.tile([128, TB], F32, tag="mps", name="mps")
                        for kc in range(4):
                            nc.tensor.matmul(
                                m_ps[:],
                                wm1_sb[:, kc, mc * 128:(mc + 1) * 128],
                                z2C[:, kc, :, :], start=(kc == 0),
                                stop=(kc == 3))
                        nc.scalar.activation(mid[:, mc, :], m_ps[:], AF.Gelu,
                                             bias=bm1_sb[:, mc:mc + 1])
                    if blk + 1 < NB:
                        z2C_cur = tr2_block(z2ts_cur, blk + 1)
                    ot = outpool.tile([128, SUBS, C], F32, tag="ot",
                                      name=f"ot{blk}")
                    for sub in range(SUBS):
                        o_ps = ops.tile([128, C], F32, tag="ops", name="ops")
                        for mc in range(16):
                            nc.tensor.matmul(
                                o_ps[:], mid[:, mc, sub * 128:(sub + 1) * 128],
                                wm2_sb[:, mc, :],
                                start=(mc == 0), stop=(mc == 15))
                        nc.vector.tensor_tensor(ot[:, sub, :], o_ps[:],
                                                x1s[sub][:], op=ALU.add)
                        if mlp2_bias:
                            nc.vector.tensor_tensor(ot[:, sub, :],
                                                    ot[:, sub, :], b2_sb[:],
                                                    op=ALU.add)
                    nc.sync.dma_start(
                        out[blk * TB:(blk + 1) * TB, :].rearrange(
                            "(s p) c -> p s c", p=128),
                        ot[:])

    nc.compile()
    return nc


_prog_cache = {}


def _get_program(T, **flags):
    key = (T, tuple(sorted(flags.items())))
    if key not in _prog_cache:
        _prog_cache[key] = build_program(T, **flags)
    return _prog_cache[key]


def make_inputs(norm1_w, norm1_b, qkv_w, qkv_b, lnk_w, lnk_b, lnv_w, lnv_b,
                proj_w, proj_b, norm2_w, norm2_b, mlp_w1, mlp_b1, mlp_w2,
                mlp_b2):
    """Host-side folds. Returns (flags, base_input_map)."""
    f8 = np.float64
    bf = ml_dtypes.bfloat16
    qkv_w = np.asarray(qkv_w, f8)
    mlp_w1 = np.asarray(mlp_w1, f8)
    wqkv_f = qkv_w * np.asarray(norm1_w, f8)[:, None]
    bqkv_f = np.asarray(norm1_b, f8) @ qkv_w + np.asarray(qkv_b, f8)
    wm1_f = mlp_w1 * np.asarray(norm2_w, f8)[:, None]
    bm1_f = np.asarray(norm2_b, f8) @ mlp_w1 + np.asarray(mlp_b1, f8)
    wproj_f = np.asarray(proj_w, f8) * SCALE

    flags = dict(
        kv_bias=bool(np.any(bqkv_f[C:] != 0.0)),
        q_bias=bool(np.any(bqkv_f[:C] != 0.0)),
        kv_affine=not (np.allclose(lnk_w, 1) and np.allclose(lnk_b, 0)
                       and np.allclose(lnv_w, 1) and np.allclose(lnv_b, 0)),
        proj_bias=bool(np.any(np.asarray(proj_b) != 0.0)),
        mlp2_bias=bool(np.any(np.asarray(mlp_b2) != 0.0)),
    )
    base = {
        "wqkv": np.ascontiguousarray(wqkv_f.astype(bf).reshape(4, 128, 3 * C)),
        "wproj": np.ascontiguousarray(wproj_f.astype(bf).reshape(H, 128, C)),
        "wm1": np.ascontiguousarray(wm1_f.astype(bf).reshape(4, 128, HID)),
        "wm2": np.ascontiguousarray(
            np.asarray(mlp_w2, f8).astype(bf).reshape(16, 128, C)),
        "bq": np.ascontiguousarray(
            bqkv_f[:C].astype(np.float32).reshape(H, 128).T),
        "bm1": np.ascontiguousarray(
            bm1_f.astype(np.float32).reshape(16, 128).T),
        "bkv": np.ascontiguousarray(bqkv_f[C:].astype(bf).reshape(1, 2 * C)),
        "lnkv": np.ascontiguousarray(
            np.stack([lnk_w, lnk_b, lnv_w, lnv_b]).astype(np.float32)),
        "bpb2": np.ascontiguousarray(
            np.stack([proj_b, mlp_b2]).astype(np.float32)),
    }
    return flags, base


def kernel(x, **params):
    global last_exec_time_ns, last_profile
    x = np.asarray(x, np.float32)
    flags, base = make_inputs(**params)
    T = N // 2
    nc = _get_program(T, **flags)
    xr = x.reshape(B, 2, T, C)
    in_maps = []
    for c in range(NCORES):
        m = dict(base)
        m["xs"] = np.ascontiguousarray(xr[c // 2, c % 2])
        in_maps.append(m)
    trace = os.environ.get("KERNEL_TRACE", "0") == "1"
    res = run_bass_kernel_spmd(nc, in_maps, list(range(NCORES)), trace=trace)
    if trace:
        last_exec_time_ns = res.exec_time_ns
        last_profile = res.profile_json
    outs = [res.results[c]["out"] for c in range(NCORES)]
    return np.concatenate(outs).reshape(B, N, C).astype(np.float32, copy=False)


# revision 29
# speedup vs baseline: 1.0025x; 1.0025x over previous
"""GalerkinBlock forward on 8 trn2 NeuronCores (Bass/Tile).

Sharding: data-parallel over tokens. Core c handles batch b=c//2, sequence
half c%2 (4096 tokens each). The linear-attention context k^T v is a sum
over tokens, so each core accumulates a partial [H,D,D] context and
all-reduces it with its pair (replica groups [[0,1],[2,3],[4,5],[6,7]]).

Per-core layout strategy (P = 128 partitions):
  - activations stream token-major ("T": [128 tokens, channels]) for all
    layernorms (bn_stats over the free axis) and channel-major
    ("C": [128 channels, tokens]) for matmul contractions; T->C conversion
    uses PE transpose-mode matmuls into PSUM + one ACT copy per sub-block.
  - All matmuls run in bf16 with fp32 PSUM accumulation.
  - LN affine folds: norm1 w/b folded into qkv_w/qkv_b, norm2 into mlp_w1,
    attention SCALE folded into proj_w (all host-side, exact).
  - attn+proj fused: x1 = x + sum_h q_h @ Watt_h with
    Watt_h = ctx_h @ (SCALE*proj_w_h) computed once after the all-reduce.
  - Both passes are software-pipelined one block ahead (the DVE/ACT
    layernorm chains for block b+1 are emitted before block b's heavy
    matmuls so the PE never starves).
"""

import os
import sys

import numpy as np

for _p in ("/opt/trn_rl_repo",):
    if os.path.isdir(_p) and _p not in sys.path:
        sys.path.insert(0, _p)

import ml_dtypes
import concourse.bass as bass
import concourse.tile as tile
from concourse import bacc, mybir
from concourse.bass_utils import run_bass_kernel_spmd
from concourse.masks import make_identity

F32 = mybir.dt.float32
BF16 = mybir.dt.bfloat16
AF = mybir.ActivationFunctionType
ALU = mybir.AluOpType

B, N, C = 4, 8192, 512
H, D = 4, 128
HID = 4 * C
EPS = 1e-5
SCALE = D ** -0.5
NCORES = 8
TB = 512          # tokens per block
SUBS = TB // 128  # 128-token sub-blocks per block

# set by the last run when KERNEL_TRACE=1
last_exec_time_ns = None
last_profile = None


def _bcast_ap(src_ap, parts, reps):
    """[K]-shaped DRAM AP -> [parts, reps*K] broadcast AP (partition step 0)."""
    ap = [[0, parts]]
    if reps > 1:
        ap.append([0, reps])
    ap.extend(src_ap.ap)
    return bass.AP(tensor=src_ap.tensor, offset=src_ap.offset, ap=ap)


def build_program(T=N // 2, kv_bias=False, kv_affine=False, proj_bias=False,
                  mlp2_bias=False, q_bias=False):
    assert T % TB == 0
    NB = T // TB
    nc = bacc.Bacc("TRN2", target_bir_lowering=False, debug=False,
                   num_devices=NCORES)

    xs = nc.dram_tensor("xs", [T, C], F32, kind="ExternalInput")
    wqkv = nc.dram_tensor("wqkv", [4, 128, 3 * C], BF16, kind="ExternalInput")
    wproj = nc.dram_tensor("wproj", [H, 128, C], BF16, kind="ExternalInput")
    wm1 = nc.dram_tensor("wm1", [4, 128, HID], BF16, kind="ExternalInput")
    wm2 = nc.dram_tensor("wm2", [16, 128, C], BF16, kind="ExternalInput")
    bq = nc.dram_tensor("bq", [128, H], F32, kind="ExternalInput")
    bm1 = nc.dram_tensor("bm1", [128, 16], F32, kind="ExternalInput")
    bkv = nc.dram_tensor("bkv", [1, 2 * C], BF16, kind="ExternalInput")
    lnkv = nc.dram_tensor("lnkv", [4, 128], F32, kind="ExternalInput")
    bpb2 = nc.dram_tensor("bpb2", [2, C], F32, kind="ExternalInput")
    out = nc.dram_tensor("out", [T, C], F32, kind="ExternalOutput")

    rg = [[0, 1], [2, 3], [4, 5], [6, 7]]

    with tile.TileContext(nc) as tc:
        from contextlib import ExitStack
        with ExitStack() as outer:
            singles = outer.enter_context(tc.tile_pool(name="singles", bufs=1))
            xpool = outer.enter_context(tc.tile_pool(name="xin", bufs=4))
            zpool = outer.enter_context(tc.tile_pool(name="zt", bufs=14))
            stats = outer.enter_context(tc.tile_pool(name="stats", bufs=4))
            trpool = outer.enter_context(
                tc.tile_pool(name="trp", bufs=1, space="PSUM"))
            dpool = outer.enter_context(
                tc.tile_pool(name="dram", bufs=1, space="DRAM"))

            # ---- persistent weights / constants ----
            # small constants first: the identity matrix gates block 0's PE
            # transposes, so it must not queue behind megabytes of weights
            eps_t = singles.tile([128, 1], F32)
            nc.vector.memset(eps_t[:], EPS)
            ident = singles.tile([128, 128], BF16)
            make_identity(nc, ident[:])
            bq_sb = singles.tile([128, H], F32)
            nc.gpsimd.dma_start(bq_sb[:], bq[:])
            bm1_sb = singles.tile([128, 16], F32)
            nc.gpsimd.dma_start(bm1_sb[:], bm1[:])
            wqkv_sb = singles.tile([128, 4, 3 * C], BF16)
            nc.gpsimd.dma_start(wqkv_sb[:, :, C:3 * C],
                                wqkv[:, :, C:3 * C].rearrange("a p n -> p a n"))
            nc.gpsimd.dma_start(wqkv_sb[:, :, 0:C],
                                wqkv[:, :, 0:C].rearrange("a p n -> p a n"))
            wproj_sb = singles.tile([128, H, C], BF16)
            nc.gpsimd.dma_start(wproj_sb[:], wproj[:].rearrange("a p n -> p a n"))
            wm1_sb = singles.tile([128, 4, HID], BF16)
            nc.gpsimd.dma_start(wm1_sb[:], wm1[:].rearrange("a p n -> p a n"))
            wm2_sb = singles.tile([128, 16, C], BF16)
            nc.gpsimd.dma_start(wm2_sb[:], wm2[:].rearrange("a p n -> p a n"))

            q_all = singles.tile([128, H, T], BF16)

            if kv_bias:
                bkv_sb = singles.tile([1, 2 * C], BF16)
                nc.gpsimd.dma_start(bkv_sb[:], bkv[:])
                ones1 = singles.tile([1, 128], BF16)
                nc.vector.memset(ones1[:], 1.0)
            if kv_affine:
                aff = []
                for r in range(4):
                    t = singles.tile([128, 4, 128], F32, tag=f"aff{r}",
                                     name=f"aff{r}")
                    nc.gpsimd.dma_start(t[:], _bcast_ap(lnkv[r, :], 128, 4))
                    aff.append(t)
                wk4, bk4, wv4, bv4 = aff
            if proj_bias:
                bp_sb = singles.tile([128, C], F32)
                nc.gpsimd.dma_start(bp_sb[:], _bcast_ap(bpb2[0, :], 128, 1))
            if mlp2_bias:
                b2_sb = singles.tile([128, C], F32)
                nc.gpsimd.dma_start(b2_sb[:], _bcast_ap(bpb2[1, :], 128, 1))

            def layernorm_stats(x_ap):
                """x_ap [128, C] f32 -> (mu [128,1], rsig [128,1])"""
                st6 = stats.tile([128, 6], F32, tag="st6", name="st6")
                nc.vector.bn_stats(st6[:], x_ap)
                mv = stats.tile([128, 2], F32, tag="mv", name="mv")
                nc.vector.bn_aggr(mv[:], st6[:])
                sd = stats.tile([128, 1], F32, tag="sd", name="sd")
                nc.scalar.activation(sd[:], mv[:, 1:2], AF.Sqrt, bias=eps_t[:])
                rs = stats.tile([128, 1], F32, tag="rs", name="rs")
                nc.vector.reciprocal(rs[:], sd[:])
                return mv[:, 0:1], rs[:]

            def transpose_sub(zt, zC, sub):
                """PE-transpose [128tok, 512ch] -> zC[:, :, sub, :]"""
                tr = trpool.tile([128, 4, 128], BF16, tag="trp", name="tr")
                for kc in range(4):
                    nc.tensor.transpose(tr[:, kc, :],
                                        zt[:, kc * 128:(kc + 1) * 128],
                                        ident[:])
                nc.scalar.activation(zC[:, :, sub, :], tr[:], AF.Copy)

            # ================= PASS 1: qkv + context =================
            # q matmuls for the last N_DEFER blocks are deferred to after the
            # context all-reduce trigger, so the collective's latency hides
            # under ~N_DEFER*3.5us of pure PE work.
            N_DEFER = NB
            with ExitStack() as p1:
                zcpool = p1.enter_context(
                    tc.tile_pool(name="zc", bufs=N_DEFER + 2))
                zkvpool = p1.enter_context(tc.tile_pool(name="zkv", bufs=4))
                # PSUM pools for the kv phase live in their own stack so the
                # deferred-q phase can reuse their banks
                p1a = ExitStack()
                ctxpool = p1a.enter_context(
                    tc.tile_pool(name="ctxps", bufs=1, space="PSUM"))
                kvps = p1a.enter_context(
                    tc.tile_pool(name="kvps", bufs=3, space="PSUM"))

                ctx_ps = ctxpool.tile([128, H, 128], F32)

                def ln1_block(blk):
                    xt = xpool.tile([128, SUBS, C], F32, tag="xt",
                                    name=f"xt{blk}")
                    nc.sync.dma_start(
                        xt[:],
                        xs[blk * TB:(blk + 1) * TB, :].rearrange(
                            "(s p) c -> p s c", p=128))
                    zts = []
                    for sub in range(SUBS):
                        mu, rs = layernorm_stats(xt[:, sub, :])
                        zt = zpool.tile([128, C], BF16, tag="zt",
                                        name=f"zt{blk}_{sub}")
                        nc.vector.tensor_scalar(
                            zt[:], xt[:, sub, :], mu, rs,
                            op0=ALU.subtract, op1=ALU.mult)
                        zts.append(zt)
                    return zts

                last_ctx_mm = [None]

                def emit_ctx(zk, zv, blk, sub):
                    for h in range(H):
                        first = (blk == 0 and sub == 0 and h == 0)
                        last = (blk == NB - 1 and sub == SUBS - 1
                                and h == H - 1)
                        last_ctx_mm[0] = nc.tensor.matmul(
                            ctx_ps[:, h, :],
                            zv[:, h * 128:(h + 1) * 128],
                            zk[:, h * 128:(h + 1) * 128],
                            start=first, stop=last)

                def emit_q(blk, zC, after=None):
                    for mc in range(H):
                        q_ps = qps.tile([128, TB], F32, tag="qps", name="qps")
                        for kc in range(4):
                            mm = nc.tensor.matmul(
                                q_ps[:],
                                wqkv_sb[:, kc, mc * 128:(mc + 1) * 128],
                                zC[:, kc, :, :], start=(kc == 0),
                                stop=(kc == 3))
                            if after is not None:
                                tile.add_dep_helper(
                                    mm.ins, after.ins, sync=False,
                                    reason="defer q past ctx accumulation")
                        if q_bias:
                            nc.scalar.activation(
                                q_all[:, mc, blk * TB:(blk + 1) * TB],
                                q_ps[:], AF.Identity,
                                bias=bq_sb[:, mc:mc + 1])
                        else:
                            nc.scalar.activation(
                                q_all[:, mc, blk * TB:(blk + 1) * TB],
                                q_ps[:], AF.Copy)

                ln_ahead = [ln1_block(b) for b in range(min(2, NB))]
                deferred_q = []
                pending = []
                for blk in range(NB):
                    zts = ln_ahead.pop(0)
                    if blk + 2 < NB:
                        ln_ahead.append(ln1_block(blk + 2))
                    zC = zcpool.tile([128, 4, SUBS, 128], BF16, tag="zC",
                                     name=f"zC{blk}")
                    for sub in range(SUBS):
                        transpose_sub(zts[sub], zC, sub)
                    # q: weight-stationary, C-major out [d, tok]
                    if blk < NB - N_DEFER:
                        emit_q(blk, zC)
                    else:
                        deferred_q.append((blk, zC))
                    # k,v: x-stationary, T-major out [tok, ch]; the ctx
                    # accumulation for sub s is emitted during sub s+2 so the
                    # PE isn't gated on s's LN-stats chain (~6us deep).
                    for sub in range(SUBS):
                        k_ps = kvps.tile([128, C], F32, tag="kps", name="kps")
                        v_ps = kvps.tile([128, C], F32, tag="vps", name="vps")
                        for kc in range(4):
                            zsub = zC[:, kc, sub, :]
                            nc.tensor.matmul(
                                k_ps[:], zsub, wqkv_sb[:, kc, C:2 * C],
                                start=(kc == 0),
                                stop=(kc == 3 and not kv_bias))
                            nc.tensor.matmul(
                                v_ps[:], zsub, wqkv_sb[:, kc, 2 * C:3 * C],
                                start=(kc == 0),
                                stop=(kc == 3 and not kv_bias))
                        if kv_bias:
                            nc.tensor.matmul(k_ps[:], ones1[:],
                                             bkv_sb[:, 0:C],
                                             start=False, stop=True)
                            nc.tensor.matmul(v_ps[:], ones1[:],
                                             bkv_sb[:, C:2 * C],
                                             start=False, stop=True)
                        if len(pending) >= 2:
                            emit_ctx(*pending.pop(0))
                        zk = zkvpool.tile([128, C], BF16, tag="zk", name="zk")
                        zv = zkvpool.tile([128, C], BF16, tag="zv", name="zv")
                        for ps, z in ((k_ps, zk), (v_ps, zv)):
                            mv = stats.tile([128, 4, 2], F32, tag="kvmv",
                                            name="kvmv")
                            for h in range(H):
                                st6 = stats.tile([128, 6], F32, tag="kvst6",
                                                 name="kvst6")
                                nc.vector.bn_stats(
                                    st6[:], ps[:, h * 128:(h + 1) * 128])
                                nc.vector.bn_aggr(mv[:, h, :], st6[:])
                            sd = stats.tile([128, 4], F32, tag="kvsd",
                                            name="kvsd")
                            nc.scalar.activation(sd[:], mv[:, :, 1], AF.Sqrt,
                                                 bias=eps_t[:])
                            rs = stats.tile([128, 4], F32, tag="kvrs",
                                            name="kvrs")
                            nc.vector.reciprocal(rs[:], sd[:])
                            nmrs = stats.tile([128, 4], F32, tag="kvnm",
                                              name="kvnm")
                            nc.vector.tensor_tensor(nmrs[:], mv[:, :, 0],
                                                    rs[:], op=ALU.mult)
                            nc.vector.tensor_scalar_mul(nmrs[:], nmrs[:], -1.0)
                            for h in range(H):
                                nc.scalar.activation(
                                    z[:, h * 128:(h + 1) * 128],
                                    ps[:, h * 128:(h + 1) * 128],
                                    AF.Identity, bias=nmrs[:, h:h + 1],
                                    scale=rs[:, h:h + 1])
                        if kv_affine:
                            for z, w4, b4 in ((zk, wk4, bk4), (zv, wv4, bv4)):
                                zf = z[:].rearrange("p (h d) -> p h d", d=128)
                                nc.vector.tensor_tensor(zf, zf, w4[:],
                                                        op=ALU.mult)
                                nc.vector.tensor_tensor(zf, zf, b4[:],
                                                        op=ALU.add)
                        pending.append((zk, zv, blk, sub))
                while pending:
                    emit_ctx(*pending.pop(0))

                ctx_sb = singles.tile([128, H * 128], BF16)
                nc.vector.tensor_copy(
                    ctx_sb[:], ctx_ps[:].rearrange("p h d -> p (h d)"))

                # launch the all-reduce (bf16: CCE adds in bf16, halves the
                # wire bytes), then run the deferred q matmuls under it
                cc_in = dpool.tile([128, H * 128], BF16)
                cc_out = dpool.tile([128, H * 128], BF16)
                nc.gpsimd.dma_start(cc_in[:], ctx_sb[:])
                nc.gpsimd.collective_compute(
                    "AllReduce", ALU.add, replica_groups=rg,
                    ins=[cc_in[:].opt()], outs=[cc_out[:].opt()])
                p1a.close()
                p1b = ExitStack()
                qps = p1b.enter_context(
                    tc.tile_pool(name="qps", bufs=2, space="PSUM"))
                for blk, zC in deferred_q:
                    emit_q(blk, zC, after=last_ctx_mm[0])
                p1b.close()

            # ================= PASS 2: attn+proj, mlp =================
            with ExitStack() as p2:
                z2cpool = p2.enter_context(tc.tile_pool(name="z2c", bufs=2))
                x1pool = p2.enter_context(tc.tile_pool(name="x1", bufs=9))
                midpool = p2.enter_context(tc.tile_pool(name="mid", bufs=2))
                outpool = p2.enter_context(tc.tile_pool(name="outp", bufs=2))
                xps = p2.enter_context(
                    tc.tile_pool(name="xps", bufs=2, space="PSUM"))
                mps = p2.enter_context(
                    tc.tile_pool(name="mps", bufs=2, space="PSUM"))
                ops = p2.enter_context(
                    tc.tile_pool(name="ops", bufs=2, space="PSUM"))

                # Watt_h = ctx_h @ (SCALE * proj_w_h)  -> [d, C] bf16
                watt_sb = singles.tile([128, H, C], BF16)
                ctxr_bf = singles.tile([128, H * 128], BF16)
                nc.gpsimd.dma_start(ctxr_bf[:], cc_out[:])
                for h in range(H):
                    w_ps = xps.tile([128, C], F32, tag="xps", name="wps")
                    nc.tensor.matmul(w_ps[:],
                                     ctxr_bf[:, h * 128:(h + 1) * 128],
                                     wproj_sb[:, h, :], start=True, stop=True)
                    nc.scalar.activation(watt_sb[:, h, :], w_ps[:], AF.Copy)

                def attn_chain(blk):
                    """attn+proj matmuls, residual, LN2 -> (x1s, z2ts)"""
                    xt = xpool.tile([128, SUBS, C], F32, tag="xt",
                                    name=f"x2t{blk}")
                    nc.sync.dma_start(
                        xt[:],
                        xs[blk * TB:(blk + 1) * TB, :].rearrange(
                            "(s p) c -> p s c", p=128))
                    x1s, z2ts = [], []
                    for sub in range(SUBS):
                        x1_ps = xps.tile([128, C], F32, tag="xps", name="xps")
                        t0 = blk * TB + sub * 128
                        for h in range(H):
                            nc.tensor.matmul(
                                x1_ps[:], q_all[:, h, t0:t0 + 128],
                                watt_sb[:, h, :], start=(h == 0),
                                stop=(h == 3))
                        x1t = x1pool.tile([128, C], F32, tag="x1",
                                          name=f"x1_{blk}_{sub}")
                        nc.vector.tensor_tensor(x1t[:], x1_ps[:],
                                                xt[:, sub, :], op=ALU.add)
                        if proj_bias:
                            nc.vector.tensor_tensor(x1t[:], x1t[:], bp_sb[:],
                                                    op=ALU.add)
                        mu, rs = layernorm_stats(x1t[:])
                        z2t = zpool.tile([128, C], BF16, tag="zt",
                                         name=f"z2t{blk}_{sub}")
                        nc.vector.tensor_scalar(
                            z2t[:], x1t[:], mu, rs,
                            op0=ALU.subtract, op1=ALU.mult)
                        x1s.append(x1t)
                        z2ts.append(z2t)
                    return x1s, z2ts

                def tr2_block(z2ts, blk):
                    z2C = z2cpool.tile([128, 4, SUBS, 128], BF16, tag="z2C",
                                       name=f"z2C{blk}")
                    for sub in range(SUBS):
                        transpose_sub(z2ts[sub], z2C, sub)
                    return z2C

                x1s_cur, z2ts_cur = attn_chain(0)
                z2C_cur = tr2_block(z2ts_cur, 0)
                for blk in range(NB):
                    x1s, z2C = x1s_cur, z2C_cur
                    if blk + 1 < NB:
                        x1s_cur, z2ts_cur = attn_chain(blk + 1)
                    mid = midpool.tile([128, 16, TB], BF16, tag="mid",
                                       name=f"mid{blk}")
                    for mc in range(16):
                        m_ps = mps.tile([128, TB], F32, tag="mps", name="mps")
                        for kc in range(4):
                            nc.tensor.matmul(
                                m_ps[:],
                                wm1_sb[:, kc, mc * 128:(mc + 1) * 128],
                                z2C[:, kc, :, :], start=(kc == 0),
                                stop=(kc == 3))
                        nc.scalar.activation(mid[:, mc, :], m_ps[:], AF.Gelu,
                                             bias=bm1_sb[:, mc:mc + 1])
                    if blk + 1 < NB:
                        z2C_cur = tr2_block(z2ts_cur, blk + 1)
                    ot = outpool.tile([128, SUBS, C], F32, tag="ot",
                                      name=f"ot{blk}")
                    for sub in range(SUBS):
                        o_ps = ops.tile([128, C], F32, tag="ops", name="ops")
                        for mc in range(16):
                            nc.tensor.matmul(
                                o_ps[:], mid[:, mc, sub * 128:(sub + 1) * 128],
                                wm2_sb[:, mc, :],
                                start=(mc == 0), stop=(mc == 15))
                        nc.vector.tensor_tensor(ot[:, sub, :], o_ps[:],
                                                x1s[sub][:], op=ALU.add)
                        if mlp2_bias:
                            nc.vector.tensor_tensor(ot[:, sub, :],
                                                    ot[:, sub, :], b2_sb[:],
                                                    op=ALU.add)
                    nc.sync.dma_start(
                        out[blk * TB:(blk + 1) * TB, :].rearrange(
                            "(s p) c -> p s c", p=128),
                        ot[:])

    nc.compile()
    return nc


_prog_cache = {}


def _get_program(T, **flags):
    key = (T, tuple(sorted(flags.items())))
    if key not in _prog_cache:
        _prog_cache[key] = build_program(T, **flags)
    return _prog_cache[key]


def make_inputs(norm1_w, norm1_b, qkv_w, qkv_b, lnk_w, lnk_b, lnv_w, lnv_b,
                proj_w, proj_b, norm2_w, norm2_b, mlp_w1, mlp_b1, mlp_w2,
                mlp_b2):
    """Host-side folds. Returns (flags, base_input_map)."""
    f8 = np.float64
    bf = ml_dtypes.bfloat16
    qkv_w = np.asarray(qkv_w, f8)
    mlp_w1 = np.asarray(mlp_w1, f8)
    wqkv_f = qkv_w * np.asarray(norm1_w, f8)[:, None]
    bqkv_f = np.asarray(norm1_b, f8) @ qkv_w + np.asarray(qkv_b, f8)
    wm1_f = mlp_w1 * np.asarray(norm2_w, f8)[:, None]
    bm1_f = np.asarray(norm2_b, f8) @ mlp_w1 + np.asarray(mlp_b1, f8)
    wproj_f = np.asarray(proj_w, f8) * SCALE

    flags = dict(
        kv_bias=bool(np.any(bqkv_f[C:] != 0.0)),
        q_bias=bool(np.any(bqkv_f[:C] != 0.0)),
        kv_affine=not (np.allclose(lnk_w, 1) and np.allclose(lnk_b, 0)
                       and np.allclose(lnv_w, 1) and np.allclose(lnv_b, 0)),
        proj_bias=bool(np.any(np.asarray(proj_b) != 0.0)),
        mlp2_bias=bool(np.any(np.asarray(mlp_b2) != 0.0)),
    )
    base = {
        "wqkv": np.ascontiguousarray(wqkv_f.astype(bf).reshape(4, 128, 3 * C)),
        "wproj": np.ascontiguousarray(wproj_f.astype(bf).reshape(H, 128, C)),
        "wm1": np.ascontiguousarray(wm1_f.astype(bf).reshape(4, 128, HID)),
        "wm2": np.ascontiguousarray(
            np.asarray(mlp_w2, f8).astype(bf).reshape(16, 128, C)),
        "bq": np.ascontiguousarray(
            bqkv_f[:C].astype(np.float32).reshape(H, 128).T),
        "bm1": np.ascontiguousarray(
            bm1_f.astype(np.float32).reshape(16, 128).T),
        "bkv": np.ascontiguousarray(bqkv_f[C:].astype(bf).reshape(1, 2 * C)),
        "lnkv": np.ascontiguousarray(
            np.stack([lnk_w, lnk_b, lnv_w, lnv_b]).astype(np.float32)),
        "bpb2": np.ascontiguousarray(
            np.stack([proj_b, mlp_b2]).astype(np.float32)),
    }
    return flags, base


def kernel(x, **params):
    global last_exec_time_ns, last_profile
    x = np.asarray(x, np.float32)
    flags, base = make_inputs(**params)
    T = N // 2
    nc = _get_program(T, **flags)
    xr = x.reshape(B, 2, T, C)
    in_maps = []
    for c in range(NCORES):
        m = dict(base)
        m["xs"] = np.ascontiguousarray(xr[c // 2, c % 2])
        in_maps.append(m)
    trace = os.environ.get("KERNEL_TRACE", "0") == "1"
    res = run_bass_kernel_spmd(nc, in_maps, list(range(NCORES)), trace=trace)
    if trace:
        last_exec_time_ns = res.exec_time_ns
        last_profile = res.profile_json
    outs = [res.results[c]["out"] for c in range(NCORES)]
    return np.concatenate(outs).reshape(B, N, C).astype(np.float32, copy=False)
